# revision 1
# baseline (speedup 1.0000x reference)
"""Multi-head attention Trainium2 kernel (8 NeuronCores, tensor-parallel over heads).

Strategy:
  - 16 heads / 8 cores = 2 heads per core. x is replicated; Wq/Wk/Wv sharded by
    head; Wp row-sharded (contraction dim). Each core computes a partial
    projection output [B*T, D]; the host sums the 8 partials (+bias).
  - On chip, all contractions need the contracted dim on SBUF partitions, so the
    host passes xT = x.reshape(BT, D).T and per-core transposed weight slices.
  - qT/kT are computed packed [128 = 2 heads x 64, BT]. Scores are computed
    transposed (s on partitions, t on free) so softmax normalization can ride
    the attn@v matmul: lhsT = [v | ones] gives out rows 0..63 = unnormalized
    out^T and row 64 = the softmax denominator Z. Softmax is computed without
    max subtraction (scores are O(10), exp stays in fp32 range).
  - Causality: only lower-triangular [128s x 512t] blocks are computed; the 4
    blocks per t-block straddling the diagonal share one [128,128] staircase
    mask (applied multiplicatively after exp) plus a memset of fully-masked
    columns.
  - Per-stage precision: SBUF tiles are all f32; stages listed in the config
    bitcast their matmul operands to float32r (TF32-like, 4x faster at N>=256).
"""

import numpy as np

B, T, D, H, HD = 2, 2048, 1024, 16, 64
NCORES = 8
HPC = H // NCORES          # heads per core = 2
CH = HPC * HD              # channels per core = 128
BT = B * T

_CACHE = {}

# which matmul stages run in float32r per named config
_CFGS = {
    "f32":  frozenset(),
    "mix":  frozenset({"v", "tr", "av", "bcast", "proj"}),
    "f32r": frozenset({"qk", "v", "tr", "scores", "av", "bcast", "proj"}),
}


def _build(b, t, d, rset):
    """Build + compile the per-core Bass program."""
    import os
    import concourse.tile as tile
    from concourse import bacc, mybir
    from concourse.masks import make_identity
    from contextlib import ExitStack

    phases = os.environ.get("KERNEL_PHASES", "123")

    f32 = mybir.dt.float32
    f32r = mybir.dt.float32r

    def mm(out, lhsT, rhs, stage, **kw):
        if stage in rset:
            lhsT = lhsT.bitcast(f32r)
            rhs = rhs.bitcast(f32r)
        nc.tensor.matmul(out, lhsT, rhs, **kw)

    bt = b * t
    KT = d // 128            # k-tiles over the model dim
    TBLK = min(512, t)       # t-block width for scores/attn
    NJ = t // TBLK           # t-blocks per batch
    NSB = bt // 128          # 128-row s-blocks over B*T
    SPT = TBLK // 128        # s-blocks per t-block

    nc = bacc.Bacc("TRN2", target_bir_lowering=False, debug=False)

    xT = nc.dram_tensor("xT", [d, bt], f32, kind="ExternalInput").ap()
    wq = nc.dram_tensor("wq", [d, CH], f32, kind="ExternalInput").ap()
    wk = nc.dram_tensor("wk", [d, CH], f32, kind="ExternalInput").ap()
    wv = nc.dram_tensor("wv", [d, CH], f32, kind="ExternalInput").ap()
    wp = nc.dram_tensor("wp", [CH, d], f32, kind="ExternalInput").ap()
    out_p = nc.dram_tensor("out_p", [bt, d], f32, kind="ExternalOutput").ap()

    with tile.TileContext(nc) as tc, ExitStack() as top:
        persist = top.enter_context(tc.tile_pool(name="persist", bufs=1))

        # ---- persistent tiles ----
        qT_sb = persist.tile([128, bt], f32, tag="qT")
        kT_sb = persist.tile([128, bt], f32, tag="kT")
        # [v_h0 | 1 | pad | v_h1 | 1 | pad] per 128-row s-block
        vaug = persist.tile([128, NSB, 66 * HPC], f32, tag="vaug")
        outT_sb = persist.tile([128, bt], f32, tag="outT")
        wq_sb = persist.tile([128, KT, CH], f32, tag="wq")
        wk_sb = persist.tile([128, KT, CH], f32, tag="wk")
        wv_sb = persist.tile([128, KT, CH], f32, tag="wv")
        wp_sb = persist.tile([128, d], f32, tag="wp")
        ident = persist.tile([128, 128], f32, tag="ident")
        mask = persist.tile([128, 128], f32, tag="mask")
        ones1 = persist.tile([65, HD], f32, tag="ones1")

        make_identity(nc, ident[:])
        nc.gpsimd.memset(vaug[:], 1.0)
        nc.gpsimd.memset(ones1[:], 1.0)
        # staircase mask: keep (p <= c), i.e. upper-triangular incl. diagonal
        nc.gpsimd.memset(mask[:], 1.0)
        nc.gpsimd.affine_select(
            out=mask[:], in_=mask[:],
            compare_op=mybir.AluOpType.is_ge,
            fill=0.0, base=0,
            # iota = -p + c ; keep when >= 0
            pattern=[[1, 128]], channel_multiplier=-1,
        )

        for w_ap, w_sb in ((wq, wq_sb), (wk, wk_sb), (wv, wv_sb)):
            nc.gpsimd.dma_start(
                out=w_sb[:],
                in_=w_ap.rearrange("(kt p) m -> p kt m", p=128),
            )
        nc.gpsimd.dma_start(out=wp_sb[:], in_=wp)

        # ---- merged loop: per (batch, t-block): QKV -> attention -> proj ----
        # Attention for block j of batch bb needs q columns of block j and
        # k/v columns of blocks 0..j (same batch) -- all computed by the time
        # block j's QKV is done, so one fused loop pipelines everything:
        # xT loads prefetch under attention PE work, and output stores drain
        # under the next block's compute.
        PW = min(512, d)
        NIB = d // PW
        with ExitStack() as body:
            xpool = body.enter_context(tc.tile_pool(name="xpool", bufs=3))
            vtpool = body.enter_context(tc.tile_pool(name="vtpool", bufs=2))
            npool = body.enter_context(tc.tile_pool(name="npool", bufs=8))
            zpool = body.enter_context(tc.tile_pool(name="zpool", bufs=2))
            tmpool = body.enter_context(tc.tile_pool(name="tmpool", bufs=2))
            opool = body.enter_context(tc.tile_pool(name="opool", bufs=4))
            # PSUM budget (8 banks): qkv 2 + scores/bcast 2 + av 2 + tr/proj 2
            ps_qkv = body.enter_context(tc.tile_pool(name="ps_qkv", bufs=2, space="PSUM"))
            ps_s = body.enter_context(tc.tile_pool(name="ps_s", bufs=2, space="PSUM"))
            ps_av = body.enter_context(tc.tile_pool(name="ps_av", bufs=2, space="PSUM"))
            ps_tp = body.enter_context(tc.tile_pool(name="ps_tp", bufs=2, space="PSUM"))

            def emit_qkv(bb, j):
                col0 = bb * t + j * TBLK
                tsl = slice(col0, col0 + TBLK)
                xt = xpool.tile([128, KT, TBLK], f32, tag="xt", name=f"xt_{bb}_{j}")
                for kt in range(KT):
                    nc.sync.dma_start(
                        out=xt[:, kt, :],
                        in_=xT[kt * 128:(kt + 1) * 128, tsl],
                    )
                for w_sb, dst, stg in ((wq_sb, qT_sb, "qk"), (wk_sb, kT_sb, "qk")):
                    ps = ps_qkv.tile([128, TBLK], f32, tag="ps_qkv",
                                     name=f"psq_{bb}_{j}_{stg}_{dst.name}")
                    for kt in range(KT):
                        mm(ps[:], w_sb[:, kt, :], xt[:, kt, :], stg,
                           start=(kt == 0), stop=(kt == KT - 1))
                    nc.vector.tensor_copy(dst[:, tsl], ps[:])
                ps = ps_qkv.tile([128, TBLK], f32, tag="ps_qkv", name=f"psv_{bb}_{j}")
                for kt in range(KT):
                    mm(ps[:], wv_sb[:, kt, :], xt[:, kt, :], "v",
                       start=(kt == 0), stop=(kt == KT - 1))
                vt = vtpool.tile([128, TBLK], f32, tag="vt", name=f"vt_{bb}_{j}")
                nc.vector.tensor_copy(vt[:], ps[:])
                for s4 in range(SPT):
                    sb_idx = (col0 // 128) + s4
                    pt = ps_tp.tile([128, 128], f32, tag="ps_tp", name=f"ptr_{bb}_{j}_{s4}")
                    vin = vt[:, s4 * 128:(s4 + 1) * 128]
                    iid = ident[:]
                    pout = pt[:]
                    if "tr" in rset:
                        vin = vin.bitcast(f32r)
                        iid = iid.bitcast(f32r)
                        pout = pout.bitcast(f32r)
                    nc.tensor.transpose(pout, vin, iid)
                    nc.vector.tensor_copy(
                        vaug[:, sb_idx, :].rearrange(
                            "p (g c) -> p g c", g=HPC)[:, :, 0:HD],
                        pt[:].rearrange("p (g c) -> p g c", g=HPC),
                    )

            def emit_attn(bb, j):
                col0 = bb * t + j * TBLK
                tsl = slice(col0, col0 + TBLK)
                n_i = (j + 1) * SPT
                avs = [ps_av.tile([65, TBLK], f32, tag="ps_av", name=f"av_{bb}_{j}_{h}")
                       for h in range(HPC)]

                def emit_av(i_, nh_pair):
                    for h in range(HPC):
                        sb_idx = (bb * t + i_ * 128) // 128
                        mm(avs[h][:], vaug[:, sb_idx, h * 66:h * 66 + HD + 1],
                           nh_pair[h][:], "av",
                           start=(i_ == 0), stop=(i_ == n_i - 1))

                nh_prev = None
                i_prev = -1
                for i in range(n_i):
                    ssl = slice(bb * t + i * 128, bb * t + i * 128 + 128)
                    dd = 128 * i - TBLK * j
                    nh_pair = []
                    for h in range(HPC):
                        hp = slice(h * HD, (h + 1) * HD)
                        ps = ps_s.tile([128, TBLK], f32, tag="ps_s",
                                       name=f"pss_{bb}_{j}_{i}_{h}")
                        mm(ps[:], kT_sb[hp, ssl], qT_sb[hp, tsl], "scores",
                           start=True, stop=True)
                        nh = npool.tile([128, TBLK], f32, tag="nh",
                                        name=f"nh_{bb}_{j}_{i}_{h}")
                        if dd < 0:
                            nc.scalar.activation(
                                nh[:], ps[:],
                                mybir.ActivationFunctionType.Exp, scale=0.125)
                        else:
                            if dd > 0:
                                nc.vector.memset(nh[:, 0:dd], 0.0)
                            nc.scalar.activation(
                                nh[:, dd:TBLK], ps[:, dd:TBLK],
                                mybir.ActivationFunctionType.Exp, scale=0.125)
                            nc.vector.tensor_mul(
                                nh[:, dd:dd + 128], nh[:, dd:dd + 128], mask[:])
                        nh_pair.append(nh)
                    # attn@v lags one i-step so exp (ACT) hides under PE
                    if nh_prev is not None:
                        emit_av(i_prev, nh_prev)
                    nh_prev, i_prev = nh_pair, i
                emit_av(i_prev, nh_prev)

                for h in range(HPC):
                    # reciprocal of Z at partition 64, then K=1 matmul
                    # broadcasts 1/Z across the 64 output partitions
                    rrow = zpool.tile([65, TBLK], f32, tag="rrow",
                                      name=f"rr_{bb}_{j}_{h}")
                    nc.vector.reciprocal(rrow[64:65, :], avs[h][64:65, :])
                    bc = ps_s.tile([HD, TBLK], f32, tag="ps_s", name=f"bc_{bb}_{j}_{h}")
                    mm(bc[:], ones1[64:65, :], rrow[64:65, :], "bcast",
                       start=True, stop=True)
                    # DVE may read only one PSUM operand: stage bc in SBUF
                    bcs = tmpool.tile([HD, TBLK], f32, tag="bcs", name=f"bcs_{bb}_{j}_{h}")
                    nc.scalar.copy(bcs[:], bc[:])
                    if h == 0:
                        nc.vector.tensor_mul(outT_sb[0:HD, tsl], avs[h][0:HD, :], bcs[:])
                    else:
                        tmp = tmpool.tile([HD, TBLK], f32, tag="tmp", name=f"tm_{bb}_{j}")
                        nc.vector.tensor_mul(tmp[:], avs[h][0:HD, :], bcs[:])
                        nc.gpsimd.dma_start(
                            out=outT_sb[h * HD:(h + 1) * HD, tsl], in_=tmp[:])

            def emit_proj(bb, j):
                col0 = bb * t + j * TBLK
                for tl in range(TBLK // 128):
                    tt = col0 // 128 + tl
                    for ib in range(NIB):
                        ps = ps_tp.tile([128, PW], f32, tag="ps_tp",
                                        name=f"psp_{bb}_{j}_{tl}_{ib}")
                        mm(ps[:], outT_sb[:, tt * 128:(tt + 1) * 128],
                           wp_sb[:, ib * PW:(ib + 1) * PW], "proj",
                           start=True, stop=True)
                        ot = opool.tile([128, PW], f32, tag="ot",
                                        name=f"ot_{bb}_{j}_{tl}_{ib}")
                        if (tl * NIB + ib) % 3 == 2:
                            nc.scalar.copy(ot[:], ps[:])
                        else:
                            nc.vector.tensor_copy(ot[:], ps[:])
                        nc.sync.dma_start(
                            out=out_p[tt * 128:(tt + 1) * 128, ib * PW:(ib + 1) * PW],
                            in_=ot[:])

            # software pipeline: QKV runs one t-block ahead of attention, and
            # the projection lags one block behind, so block-boundary DVE/DMA
            # latencies hide under attention PE work
            blocks = [(bb, j) for bb in range(b) for j in range(NJ)]
            emit_qkv(*blocks[0])
            for idx, blk in enumerate(blocks):
                if idx + 1 < len(blocks):
                    emit_qkv(*blocks[idx + 1])
                emit_attn(*blk)
                if idx >= 1:
                    emit_proj(*blocks[idx - 1])
            emit_proj(*blocks[-1])

    nc.compile()
    return nc


def _get_nc(b=B, t=T, d=D, cfg="f32"):
    key = (b, t, d, cfg)
    if key not in _CACHE:
        _CACHE[key] = _build(b, t, d, _CFGS[cfg])
    return _CACHE[key]


def _prepare_in_maps(x, Wq, Wk, Wv, Wp, b, t, d, n_heads):
    bt = b * t
    xT = np.ascontiguousarray(x.reshape(bt, d).T.astype(np.float32))
    in_maps = []
    for c in range(NCORES):
        h0 = c * HPC
        wq_c = np.ascontiguousarray(Wq[h0:h0 + HPC].reshape(CH, d).T.astype(np.float32))
        wk_c = np.ascontiguousarray(Wk[h0:h0 + HPC].reshape(CH, d).T.astype(np.float32))
        wv_c = np.ascontiguousarray(Wv[h0:h0 + HPC].reshape(CH, d).T.astype(np.float32))
        wp_c = np.ascontiguousarray(Wp[:, c * CH:(c + 1) * CH].T.astype(np.float32))
        in_maps.append({"xT": xT, "wq": wq_c, "wk": wk_c, "wv": wv_c, "wp": wp_c})
    return in_maps


def _run(x, Wq, Wk, Wv, Wp, bp, b, t, d, cfg, trace=False):
    from concourse.bass_utils import run_bass_kernel_spmd
    nc = _get_nc(b, t, d, cfg)
    in_maps = _prepare_in_maps(x, Wq, Wk, Wv, Wp, b, t, d, H)
    res = run_bass_kernel_spmd(nc, in_maps, core_ids=list(range(NCORES)), trace=trace)
    acc = np.zeros((b * t, d), dtype=np.float64)
    for r in res.results:
        acc += r["out_p"].astype(np.float64)
    out = (acc + np.asarray(bp, dtype=np.float64)).astype(np.float32)
    return out.reshape(b, t, d), res


KERNEL_CFG = "f32"


def kernel(x, Wq, Wk, Wv, Wp, bp):
    out, _ = _run(np.asarray(x), np.asarray(Wq), np.asarray(Wk), np.asarray(Wv),
                  np.asarray(Wp), np.asarray(bp), B, T, D, KERNEL_CFG, trace=False)
    return out



# revision 2
# speedup vs baseline: 3.1772x; 3.1772x over previous
"""Multi-head attention Trainium2 kernel (8 NeuronCores, tensor-parallel over heads).

Strategy:
  - 16 heads / 8 cores = 2 heads per core. x is replicated; Wq/Wk/Wv sharded by
    head; Wp row-sharded (contraction dim). Each core computes a partial
    projection output [B*T, D]; the host sums the 8 partials (+bias).
  - On chip, all contractions need the contracted dim on SBUF partitions, so the
    host passes xT = x.reshape(BT, D).T and per-core transposed weight slices.
  - qT/kT are computed packed [128 = 2 heads x 64, BT]. Scores are computed
    transposed (s on partitions, t on free) so softmax normalization can ride
    the attn@v matmul: lhsT = [v | ones] gives out rows 0..63 = unnormalized
    out^T and row 64 = the softmax denominator Z. Softmax is computed without
    max subtraction (scores are O(10), exp stays in fp32 range).
  - Causality: only lower-triangular [128s x 512t] blocks are computed; blocks
    straddling the diagonal narrow their scores/av matmuls to the unmasked
    column range [dd:TBLK] and apply one shared [128,128] staircase mask
    (multiplicative, after exp).
  - Precision: every matmul operand lives in a float32r (TF32-like) tile.
    The BIR verifier requires f32r matmul inputs to be *produced* rounded, so
    DRAM inputs are declared f32r (DMA is a byte copy; the PE rounds on read)
    and on-chip producers (PSUM->SBUF copies, exp, muls) write f32r directly.
    Memset/iota on f32r tiles fails ISA codegen, so constants are built in f32
    scratch and round-copied. f32r matmul is 4x faster than f32 at N>=256.
"""

import numpy as np

B, T, D, H, HD = 2, 2048, 1024, 16, 64
NCORES = 8
HPC = H // NCORES          # heads per core = 2
CH = HPC * HD              # channels per core = 128
BT = B * T

_CACHE = {}


def _build(b, t, d, use_f32r):
    """Build + compile the per-core Bass program."""
    import concourse.tile as tile
    from concourse import bacc, mybir
    from concourse.masks import make_identity
    from contextlib import ExitStack

    f32 = mybir.dt.float32
    dtt = mybir.dt.float32r if use_f32r else f32

    bt = b * t
    KT = d // 128            # k-tiles over the model dim
    TBLK = min(512, t)       # t-block width for scores/attn
    NJ = t // TBLK           # t-blocks per batch
    NSB = bt // 128          # 128-row s-blocks over B*T
    SPT = TBLK // 128        # s-blocks per t-block

    nc = bacc.Bacc("TRN2", target_bir_lowering=False, debug=False)

    xT = nc.dram_tensor("xT", [d, bt], dtt, kind="ExternalInput").ap()
    wq = nc.dram_tensor("wq", [d, CH], dtt, kind="ExternalInput").ap()
    wk = nc.dram_tensor("wk", [d, CH], dtt, kind="ExternalInput").ap()
    wv = nc.dram_tensor("wv", [d, CH], dtt, kind="ExternalInput").ap()
    wp = nc.dram_tensor("wp", [CH, d], dtt, kind="ExternalInput").ap()
    out_p = nc.dram_tensor("out_p", [bt, d], f32, kind="ExternalOutput").ap()

    with tile.TileContext(nc) as tc, ExitStack() as top:
        persist = top.enter_context(tc.tile_pool(name="persist", bufs=1))

        # ---- persistent tiles ----
        qT_sb = persist.tile([128, bt], dtt, tag="qT")
        kT_sb = persist.tile([128, bt], dtt, tag="kT")
        # [v_h0 | 1 | pad | v_h1 | 1 | pad] per 128-row s-block
        vaug = persist.tile([128, NSB, 66 * HPC], dtt, tag="vaug")
        outT_sb = persist.tile([128, bt], dtt, tag="outT")
        wq_sb = persist.tile([128, KT, CH], dtt, tag="wq")
        wk_sb = persist.tile([128, KT, CH], dtt, tag="wk")
        wv_sb = persist.tile([128, KT, CH], dtt, tag="wv")
        wp_sb = persist.tile([128, d], dtt, tag="wp")
        ident = persist.tile([128, 128], dtt, tag="ident")
        mask = persist.tile([128, 128], dtt, tag="mask")
        ones1 = persist.tile([65, HD], dtt, tag="ones1")

        # constants are built in f32 scratch (memset/iota on f32r fails ISA
        # codegen) and round-copied into their f32r homes
        with tc.tile_pool(name="const", bufs=1) as cpool:
            ident_f = cpool.tile([128, 128], f32, tag="ident_f")
            mask_f = cpool.tile([128, 128], f32, tag="mask_f")
            ones_f = cpool.tile([128, HD], f32, tag="ones_f")

            make_identity(nc, ident_f[:])
            nc.gpsimd.memset(ones_f[:], 1.0)
            # staircase mask: keep (p <= c), i.e. upper-triangular incl. diag
            nc.gpsimd.memset(mask_f[:], 1.0)
            nc.gpsimd.affine_select(
                out=mask_f[:], in_=mask_f[:],
                compare_op=mybir.AluOpType.is_ge,
                fill=0.0, base=0,
                # iota = -p + c ; keep when >= 0
                pattern=[[1, 128]], channel_multiplier=-1,
            )

            nc.vector.tensor_copy(ident[:], ident_f[:])
            nc.vector.tensor_copy(mask[:], mask_f[:])
            nc.vector.tensor_copy(ones1[:], ones_f[0:65, :])
            # ones column of vaug (col 64 of each 66-wide head group)
            for sb in range(NSB):
                nc.vector.tensor_copy(
                    vaug[:, sb, :].rearrange(
                        "p (g c) -> p g c", g=HPC)[:, :, 64:65],
                    ones_f[:, 0:HPC].rearrange("p (g c) -> p g c", g=HPC),
                )

        for w_ap, w_sb in ((wq, wq_sb), (wk, wk_sb), (wv, wv_sb)):
            nc.gpsimd.dma_start(
                out=w_sb[:],
                in_=w_ap.rearrange("(kt p) m -> p kt m", p=128),
            )
        nc.gpsimd.dma_start(out=wp_sb[:], in_=wp)

        # ---- merged loop: per (batch, t-block): QKV -> attention -> proj ----
        # Attention for block j of batch bb needs q columns of block j and
        # k/v columns of blocks 0..j (same batch) -- all computed by the time
        # block j's QKV is done, so one fused loop pipelines everything:
        # xT loads prefetch under attention PE work, and output stores drain
        # under the next block's compute.
        PW = min(512, d)
        NIB = d // PW
        with ExitStack() as body:
            xpool = body.enter_context(tc.tile_pool(name="xpool", bufs=3))
            vtpool = body.enter_context(tc.tile_pool(name="vtpool", bufs=2))
            npool = body.enter_context(tc.tile_pool(name="npool", bufs=8))
            zpool = body.enter_context(tc.tile_pool(name="zpool", bufs=2))
            tmpool = body.enter_context(tc.tile_pool(name="tmpool", bufs=2))
            opool = body.enter_context(tc.tile_pool(name="opool", bufs=4))
            # PSUM budget (8 banks): qkv 2 + scores/bcast 2 + av 2 + tr/proj 2
            ps_qkv = body.enter_context(tc.tile_pool(name="ps_qkv", bufs=2, space="PSUM"))
            ps_s = body.enter_context(tc.tile_pool(name="ps_s", bufs=2, space="PSUM"))
            ps_av = body.enter_context(tc.tile_pool(name="ps_av", bufs=2, space="PSUM"))
            ps_tp = body.enter_context(tc.tile_pool(name="ps_tp", bufs=2, space="PSUM"))

            def emit_qkv(bb, j):
                col0 = bb * t + j * TBLK
                tsl = slice(col0, col0 + TBLK)
                xt = xpool.tile([128, KT, TBLK], dtt, tag="xt", name=f"xt_{bb}_{j}")
                for kt in range(KT):
                    nc.sync.dma_start(
                        out=xt[:, kt, :],
                        in_=xT[kt * 128:(kt + 1) * 128, tsl],
                    )
                for w_sb, dst in ((wq_sb, qT_sb), (wk_sb, kT_sb)):
                    ps = ps_qkv.tile([128, TBLK], f32, tag="ps_qkv",
                                     name=f"psq_{bb}_{j}_{dst.name}")
                    for kt in range(KT):
                        nc.tensor.matmul(ps[:], w_sb[:, kt, :], xt[:, kt, :],
                                         start=(kt == 0), stop=(kt == KT - 1))
                    nc.vector.tensor_copy(dst[:, tsl], ps[:])
                ps = ps_qkv.tile([128, TBLK], f32, tag="ps_qkv", name=f"psv_{bb}_{j}")
                for kt in range(KT):
                    nc.tensor.matmul(ps[:], wv_sb[:, kt, :], xt[:, kt, :],
                                     start=(kt == 0), stop=(kt == KT - 1))
                vt = vtpool.tile([128, TBLK], dtt, tag="vt", name=f"vt_{bb}_{j}")
                nc.vector.tensor_copy(vt[:], ps[:])
                for s4 in range(SPT):
                    sb_idx = (col0 // 128) + s4
                    pt = ps_tp.tile([128, 128], f32, tag="ps_tp", name=f"ptr_{bb}_{j}_{s4}")
                    pout = pt[:].bitcast(dtt) if use_f32r else pt[:]
                    nc.tensor.transpose(pout, vt[:, s4 * 128:(s4 + 1) * 128], ident[:])
                    nc.vector.tensor_copy(
                        vaug[:, sb_idx, :].rearrange(
                            "p (g c) -> p g c", g=HPC)[:, :, 0:HD],
                        pt[:].rearrange("p (g c) -> p g c", g=HPC),
                    )

            def emit_attn(bb, j):
                col0 = bb * t + j * TBLK
                n_i = (j + 1) * SPT
                avs = [ps_av.tile([65, TBLK], f32, tag="ps_av", name=f"av_{bb}_{j}_{h}")
                       for h in range(HPC)]

                def emit_av(i_, nh_pair, dd_):
                    lo = max(dd_, 0)
                    for h in range(HPC):
                        sb_idx = (bb * t + i_ * 128) // 128
                        nc.tensor.matmul(
                            avs[h][:, lo:TBLK],
                            vaug[:, sb_idx, h * 66:h * 66 + HD + 1],
                            nh_pair[h][:, lo:TBLK],
                            start=(i_ == 0), stop=(i_ == n_i - 1),
                            skip_group_check=True)

                nh_prev = None
                i_prev = -1
                dd_prev = 0
                for i in range(n_i):
                    ssl = slice(bb * t + i * 128, bb * t + i * 128 + 128)
                    dd = 128 * i - TBLK * j
                    lo = max(dd, 0)
                    nh_pair = []
                    for h in range(HPC):
                        hp = slice(h * HD, (h + 1) * HD)
                        ps = ps_s.tile([128, TBLK], f32, tag="ps_s",
                                       name=f"pss_{bb}_{j}_{i}_{h}")
                        nc.tensor.matmul(
                            ps[:, lo:TBLK], kT_sb[hp, ssl],
                            qT_sb[hp, col0 + lo:col0 + TBLK],
                            start=True, stop=True)
                        nh = npool.tile([128, TBLK], dtt, tag="nh",
                                        name=f"nh_{bb}_{j}_{i}_{h}")
                        nc.scalar.activation(
                            nh[:, lo:TBLK], ps[:, lo:TBLK],
                            mybir.ActivationFunctionType.Exp, scale=0.125)
                        if dd >= 0:
                            nc.vector.tensor_mul(
                                nh[:, dd:dd + 128], nh[:, dd:dd + 128], mask[:])
                        nh_pair.append(nh)
                    # attn@v lags one i-step so exp (ACT) hides under PE
                    if nh_prev is not None:
                        emit_av(i_prev, nh_prev, dd_prev)
                    nh_prev, i_prev, dd_prev = nh_pair, i, dd
                emit_av(i_prev, nh_prev, dd_prev)

                for h in range(HPC):
                    # reciprocal of Z at partition 64, then K=1 matmul
                    # broadcasts 1/Z across the 64 output partitions
                    rrow = zpool.tile([65, TBLK], dtt, tag="rrow",
                                      name=f"rr_{bb}_{j}_{h}")
                    with nc.allow_low_precision(reason="1/Z broadcast via f32r matmul"):
                        nc.vector.reciprocal(rrow[64:65, :], avs[h][64:65, :])
                    bc = ps_s.tile([HD, TBLK], f32, tag="ps_s", name=f"bc_{bb}_{j}_{h}")
                    nc.tensor.matmul(bc[:], ones1[64:65, :], rrow[64:65, :],
                                     start=True, stop=True)
                    # DVE may read only one PSUM operand: stage bc in SBUF
                    bcs = tmpool.tile([HD, TBLK], f32, tag="bcs", name=f"bcs_{bb}_{j}_{h}")
                    nc.scalar.copy(bcs[:], bc[:])
                    tsl = slice(col0, col0 + TBLK)
                    if h == 0:
                        nc.vector.tensor_mul(outT_sb[0:HD, tsl], avs[h][0:HD, :], bcs[:])
                    else:
                        tmp = tmpool.tile([HD, TBLK], dtt, tag="tmp", name=f"tm_{bb}_{j}")
                        nc.vector.tensor_mul(tmp[:], avs[h][0:HD, :], bcs[:])
                        nc.gpsimd.dma_start(
                            out=outT_sb[h * HD:(h + 1) * HD, tsl], in_=tmp[:])

            def emit_proj(bb, j):
                col0 = bb * t + j * TBLK
                for tl in range(TBLK // 128):
                    tt = col0 // 128 + tl
                    for ib in range(NIB):
                        ps = ps_tp.tile([128, PW], f32, tag="ps_tp",
                                        name=f"psp_{bb}_{j}_{tl}_{ib}")
                        nc.tensor.matmul(ps[:], outT_sb[:, tt * 128:(tt + 1) * 128],
                                         wp_sb[:, ib * PW:(ib + 1) * PW],
                                         start=True, stop=True)
                        ot = opool.tile([128, PW], f32, tag="ot",
                                        name=f"ot_{bb}_{j}_{tl}_{ib}")
                        if (tl * NIB + ib) % 3 == 2:
                            nc.scalar.copy(ot[:], ps[:])
                        else:
                            nc.vector.tensor_copy(ot[:], ps[:])
                        nc.sync.dma_start(
                            out=out_p[tt * 128:(tt + 1) * 128, ib * PW:(ib + 1) * PW],
                            in_=ot[:])

            # software pipeline: QKV runs one t-block ahead of attention, and
            # the projection lags one block behind, so block-boundary DVE/DMA
            # latencies hide under attention PE work
            blocks = [(bb, j) for bb in range(b) for j in range(NJ)]
            emit_qkv(*blocks[0])
            for idx, blk in enumerate(blocks):
                if idx + 1 < len(blocks):
                    emit_qkv(*blocks[idx + 1])
                emit_attn(*blk)
                if idx >= 1:
                    emit_proj(*blocks[idx - 1])
            emit_proj(*blocks[-1])

    nc.compile()
    return nc


def _get_nc(b=B, t=T, d=D, cfg="f32r"):
    key = (b, t, d, cfg)
    if key not in _CACHE:
        _CACHE[key] = _build(b, t, d, cfg == "f32r")
    return _CACHE[key]


def _prepare_in_maps(x, Wq, Wk, Wv, Wp, b, t, d, n_heads):
    bt = b * t
    xT = np.ascontiguousarray(x.reshape(bt, d).T.astype(np.float32))
    in_maps = []
    for c in range(NCORES):
        h0 = c * HPC
        wq_c = np.ascontiguousarray(Wq[h0:h0 + HPC].reshape(CH, d).T.astype(np.float32))
        wk_c = np.ascontiguousarray(Wk[h0:h0 + HPC].reshape(CH, d).T.astype(np.float32))
        wv_c = np.ascontiguousarray(Wv[h0:h0 + HPC].reshape(CH, d).T.astype(np.float32))
        wp_c = np.ascontiguousarray(Wp[:, c * CH:(c + 1) * CH].T.astype(np.float32))
        in_maps.append({"xT": xT, "wq": wq_c, "wk": wk_c, "wv": wv_c, "wp": wp_c})
    return in_maps


def _run(x, Wq, Wk, Wv, Wp, bp, b, t, d, cfg, trace=False):
    from concourse.bass_utils import run_bass_kernel_spmd
    nc = _get_nc(b, t, d, cfg)
    in_maps = _prepare_in_maps(x, Wq, Wk, Wv, Wp, b, t, d, H)
    res = run_bass_kernel_spmd(nc, in_maps, core_ids=list(range(NCORES)), trace=trace)
    acc = np.zeros((b * t, d), dtype=np.float64)
    for r in res.results:
        acc += r["out_p"].astype(np.float64)
    out = (acc + np.asarray(bp, dtype=np.float64)).astype(np.float32)
    return out.reshape(b, t, d), res


KERNEL_CFG = "f32r"


def kernel(x, Wq, Wk, Wv, Wp, bp):
    out, _ = _run(np.asarray(x), np.asarray(Wq), np.asarray(Wk), np.asarray(Wv),
                  np.asarray(Wp), np.asarray(bp), B, T, D, KERNEL_CFG, trace=False)
    return out
